# revision 1
# baseline (speedup 1.0000x reference)
"""DeepEMD Trainium2 kernel: batched 49x49 entropic-OT (Sinkhorn) similarity.

Strategy (8 NeuronCores, data-parallel over batch):
- Each core gets 128 batches. Host prepacks, per (chunk j of 128 channels,
  batch b), an augmented matrix A = [Q | P | 1] (128 x 99) in fp16 (10-bit
  mantissa keeps the end-to-end logits error ~2e-4), sequenced in DRAM so
  every load DMA reads one contiguous span.
- PE computes the Gram G_b = A^T A (99x99, fp32 PSUM) with one accumulating
  fp16 matmul per channel chunk (weights widened to 128 cols to engage
  fast-weight-load; junk rows ignored). G contains Q^T P, P^T Q, column
  sums (ones row) and diag blocks -> the similarity map, norms and weight
  vectors are all cheap fixups of G.
- A per-batch SBUF->SBUF DMA flattens G_b into row b of a [128, 99*99]
  tile: everything after that runs batch-on-partitions, full 128-lane DVE.
- Sinkhorn runs in the *linear* domain (K = exp((sim-1)/eps + 16)) with
  Gauss-Seidel updates us = r/(K vs), vs = c/(K^T us). The reference's 100
  log-domain iterations are converged ~1e-12 by 20; ITERS linear f32
  iterations reach ~2e-5 at 8.
- logits[b] = T * sum(flow * sim) = T * us^T ((K.sim) vs).
- One dma_start runs on a single SDMA engine (~27 GB/s), so loads are split
  into sub-DMAs across both HWDGE queues and flattens are spread across
  gpsimd/sync/scalar issuers to keep many engines streaming.
"""

import os
import sys

import numpy as np

sys.path.insert(0, "/opt/trn_rl_repo")

import concourse.bass as bass
import concourse.bacc as bacc
import concourse.mybir as mybir
from concourse import tile
from concourse.bass_utils import run_bass_kernel_spmd

import ml_dtypes

B_FULL, C, HW = 1024, 512, 49
NCORE = 8
BS = B_FULL // NCORE  # 128 batches per core
NCH = C // 128  # 4 chunks of 128 channels (PE contraction dim)
AC = 2 * HW + 1  # 99 augmented columns [Q | P | 1]
GRP = 16  # batches per DMA group
NGRP = BS // GRP
ITERS = 6
EPS_S = 0.05
TEMP = 12.5 / HW
EXP_BIAS = -4.0  # exp((sim-1)/eps) * e^16 rescale; cancels in us*K*vs

f32 = mybir.dt.float32
f16 = mybir.dt.float16
Alu = mybir.AluOpType
Act = mybir.ActivationFunctionType
AxX = mybir.AxisListType.X


def build_nc(debug=False):
    nc = bacc.Bacc(None, target_bir_lowering=False, debug=debug)
    JW = GRP * AC  # cols per chunk-slab in stage
    aug = nc.declare_dram_parameter(
        "aug", [NGRP, 128, NCH * JW], f16, isOutput=False
    )
    outp = nc.declare_dram_parameter("out", [BS, 1], f32, isOutput=True)

    FW = AC * AC  # 9801 flat row width

    with tile.TileContext(nc) as tc:
        with (
            tc.tile_pool(name="big", bufs=1) as big,
            tc.tile_pool(name="stage", bufs=4) as stg,
            tc.tile_pool(name="gcopy", bufs=8) as gcp,
            tc.tile_pool(name="work", bufs=3) as wrk,
            tc.tile_pool(name="small", bufs=1) as sml,
            tc.tile_pool(name="psum", bufs=8, space="PSUM") as pp,
        ):
            flatG = big.tile([BS, FW], f32, tag="flatG", name="flatG")

            # ---------------- Phase 1: DMA in + Gram + flatten ----------------
            NSPL = 8
            SW = NCH * JW // NSPL
            for g in range(NGRP):
                th = stg.tile([128, NCH * JW], f16, tag="h", name="hg")
                # loads live alone on the SP queue: a flatten on the same
                # FIFO queue would head-of-line block the next group's
                # prefetch behind compute
                for ss in range(NSPL):
                    nc.sync.dma_start(
                        th[:, ss * SW : (ss + 1) * SW],
                        aug[g, :, ss * SW : (ss + 1) * SW],
                    )
                for bb in range(GRP):
                    b = g * GRP + bb
                    ps = pp.tile([128, AC], f32, tag="gram", name="gram")
                    for j in range(NCH):
                        base = j * JW + bb * AC
                        # widen weights to 128 cols (spill into following slab
                        # data -> junk G rows 99..127, never read); the very
                        # last slab position must stay 99 wide
                        wid = AC if (bb == GRP - 1 and j == NCH - 1) else 128
                        nc.tensor.matmul(
                            ps[0:wid, :],
                            th[:, base : base + wid],
                            th[:, base : base + AC],
                            start=(j == 0),
                            stop=(j == NCH - 1),
                        )
                    gs = gcp.tile([AC, AC], f32, tag="gs", name="gs")
                    nc.vector.tensor_copy(gs[:], ps[0:AC, :])
                    # flatten [99, 99] -> one batch-major row; spread issue
                    # cost across gpsimd (SWDGE) + both HWDGE queues
                    dmae = (nc.gpsimd, nc.scalar)[b % 2]
                    dmae.dma_start(flatG[b : b + 1, :], gs[:])

            # ---------------- Phase 1.5: fixup to sim/K/marginals -------------
            G3 = flatG[:].rearrange("p (q c) -> p q c", c=AC)
            qtp = G3[:, 0:HW, HW : 2 * HW]  # [128, 49, 49] raw Q^T P
            ptq = G3[:, HW : 2 * HW, 0:HW]
            sq = flatG[:, (AC - 1) * AC : (AC - 1) * AC + HW]  # 1^T Q
            sp = flatG[:, (AC - 1) * AC + HW : (AC - 1) * AC + 2 * HW]  # 1^T P

            def dview(row0, col0):
                # [128, 49] diagonal view: (row0+m)*99 + col0+m, stride 100
                v = flatG[:, row0 * AC + col0 : row0 * AC + col0 + 1].copy()
                v.ap = mybir.VecI64Pair([list(v.ap[0])] + [[AC + 1, HW]])
                return v

            dq = dview(0, 0)  # diag(QtQ)
            dp = dview(HW, HW)  # diag(PtP)

            def s49(tag):
                return sml.tile([BS, HW], f32, tag=tag, name=tag)

            inq, inp_, t1, t2 = s49("inq"), s49("inp"), s49("t1"), s49("t2")
            aq, ap_ = s49("aq"), s49("ap")
            w1, w2, us, vs = s49("w1"), s49("w2"), s49("us"), s49("vs")
            kv, rkv = s49("kv"), s49("rkv")
            s2 = sml.tile([BS, 1], f32, tag="s2", name="s2")
            ebias = sml.tile([BS, 1], f32, tag="ebias", name="ebias")
            nc.vector.memset(ebias[:], EXP_BIAS)
            # warm the ACT sqrt/exp table sets early (no data deps -> Tile
            # schedules these under the phase-1 DMA shadow while ACT is idle,
            # hiding the ~2.7us-per-set PSEUDO_LOAD_ACT_FUNC_SET cost)
            wrm = sml.tile([BS, 1], f32, tag="wrm", name="wrm")
            nc.vector.memset(wrm[:], 1.0)
            nc.scalar.activation(wrm[:], wrm[:], Act.Sqrt)
            nc.scalar.activation(wrm[:], wrm[:], Act.Exp)
            lg = sml.tile([BS, 1], f32, tag="lg", name="lg")
            lgf = sml.tile([BS, 1], f32, tag="lgf", name="lgf")

            def v3(t):  # [128, 49, 49] view of a [128, 2401] tile
                return t[:].rearrange("p (q c) -> p q c", c=HW)

            def v3t(t):  # transposed view (strides 1, 49)
                return t[:].rearrange("p (q c) -> p c q", c=HW)

            # weight vectors: w = relu(rowsum/49) + 0.001 (unnormalized: the
            # r-normalization cancels in the logits, the c-normalization is a
            # final 1/s2 scale)
            nc.vector.tensor_reduce(w1[:], qtp, axis=AxX, op=Alu.add)
            nc.vector.tensor_reduce(w2[:], ptq, axis=AxX, op=Alu.add)
            for w in (w1, w2):
                nc.vector.tensor_scalar(w[:], w[:], 1.0 / HW, 0.0, Alu.mult, Alu.max)
                nc.vector.tensor_scalar(w[:], w[:], 0.001, None, Alu.add)
            nc.vector.tensor_reduce(s2[:], w2[:], axis=AxX, op=Alu.add)

            for (sx, dx, inv) in ((sq, dq, inq), (sp, dp, inp_)):
                # u = diag - s^2/C ; inv = rsqrt(u) via sqrt LUT+recip+Newton
                nc.vector.tensor_mul(t1[:], sx, sx)
                nc.vector.scalar_tensor_tensor(
                    t2[:], t1[:], -1.0 / C, dx, Alu.mult, Alu.add
                )
                nc.scalar.activation(t1[:], t2[:], Act.Sqrt)
                nc.vector.reciprocal(inv[:], t1[:])
                nc.vector.tensor_mul(t1[:], inv[:], inv[:])
                nc.vector.tensor_mul(t1[:], t1[:], t2[:])
                nc.vector.tensor_scalar(t1[:], t1[:], -0.5, 1.5, Alu.mult, Alu.add)
                nc.vector.tensor_mul(inv[:], inv[:], t1[:])

            rC = 1.0 / np.sqrt(float(C))
            nc.vector.scalar_tensor_tensor(
                aq[:], sq, rC, inq[:], Alu.mult, Alu.mult
            )
            nc.vector.scalar_tensor_tensor(
                ap_[:], sp, rC, inp_[:], Alu.mult, Alu.mult
            )

            simb = big.tile([BS, HW * HW], f32, tag="sim", name="sim")
            Kb = big.tile([BS, HW * HW], f32, tag="K", name="K")
            Ktb = big.tile([BS, HW * HW], f32, tag="Kt", name="Kt")
            b1 = wrk.tile([BS, HW * HW], f32, tag="w", name="b1")
            b3 = wrk.tile([BS, HW * HW], f32, tag="w", name="b3")
            simTb = wrk.tile([BS, HW * HW], f32, tag="w", name="simTb")

            bq = inq[:].unsqueeze(2).broadcast_to([BS, HW, HW])
            bp = inp_[:].unsqueeze(1).broadcast_to([BS, HW, HW])
            nc.vector.tensor_mul(v3(b1), bq, bp)  # B1 = inq x inp
            nc.vector.tensor_mul(v3(simb), qtp, v3(b1))  # B2
            baq = aq[:].unsqueeze(2).broadcast_to([BS, HW, HW])
            bap = ap_[:].unsqueeze(1).broadcast_to([BS, HW, HW])
            nc.vector.tensor_mul(v3(b3), baq, bap)  # B3 = aq x ap
            nc.vector.tensor_sub(v3(simb), v3(simb), v3(b3))  # sim = B2 - B3
            nc.vector.tensor_mul(v3(simTb), ptq, v3t(b1))
            nc.vector.tensor_sub(v3(simTb), v3(simTb), v3t(b3))
            nc.scalar.activation(
                Kb[:], simb[:], Act.Exp, scale=1.0 / EPS_S, bias=ebias[:]
            )
            nc.scalar.activation(
                Ktb[:], simTb[:], Act.Exp, scale=1.0 / EPS_S, bias=ebias[:]
            )

            # ---------------- Phase 2: Sinkhorn (Gauss-Seidel, linear) --------
            tb = wrk.tile([BS, HW * HW], f32, tag="w", name="tb")
            bvs = vs[:].unsqueeze(1).broadcast_to([BS, HW, HW])
            bus = us[:].unsqueeze(1).broadcast_to([BS, HW, HW])
            for it in range(ITERS):
                if it == 0:
                    nc.vector.tensor_reduce(kv[:], v3(Kb), axis=AxX, op=Alu.add)
                else:
                    nc.vector.tensor_mul(v3(tb), v3(Kb), bvs)
                    nc.vector.tensor_reduce(kv[:], v3(tb), axis=AxX, op=Alu.add)
                nc.vector.reciprocal(rkv[:], kv[:])
                nc.vector.tensor_mul(us[:], w1[:], rkv[:])
                nc.vector.tensor_mul(v3(tb), v3(Ktb), bus)
                nc.vector.tensor_reduce(kv[:], v3(tb), axis=AxX, op=Alu.add)
                nc.vector.reciprocal(rkv[:], kv[:])
                nc.vector.tensor_mul(vs[:], w2[:], rkv[:])

            # ---------------- Phase 3: logits ---------------------------------
            nc.vector.tensor_mul(v3(tb), v3(Kb), bvs)
            nc.vector.tensor_mul(tb[:], tb[:], simb[:])
            nc.vector.tensor_reduce(kv[:], v3(tb), axis=AxX, op=Alu.add)
            nc.vector.tensor_mul(kv[:], kv[:], us[:])
            nc.vector.tensor_reduce(lg[:], kv[:], axis=AxX, op=Alu.add)
            nc.vector.reciprocal(rkv[:, 0:1], s2[:])
            nc.vector.scalar_tensor_tensor(
                lgf[:], lg[:], TEMP, rkv[:, 0:1], Alu.mult, Alu.mult
            )  # (lg * T) / s2
            nc.sync.dma_start(outp[:, :], lgf[:])

    nc.compile()
    return nc


_NC = None


def _get_nc():
    global _NC
    if _NC is None:
        _NC = build_nc()
    return _NC


def _prep_in_maps(feature_map1, feature_map2):
    q = np.ascontiguousarray(np.asarray(feature_map1, dtype=np.float32)).reshape(
        B_FULL, C, HW
    )
    p = np.ascontiguousarray(np.asarray(feature_map2, dtype=np.float32)).reshape(
        B_FULL, C, HW
    )
    in_maps = []
    for i in range(NCORE):
        sl = slice(i * BS, (i + 1) * BS)
        a32 = np.empty((NCH, 128, BS, AC), np.float32)
        a32[..., AC - 1] = 1.0
        a32[..., 0:HW] = q[sl].reshape(BS, NCH, 128, HW).transpose(1, 2, 0, 3)
        a32[..., HW : 2 * HW] = p[sl].reshape(BS, NCH, 128, HW).transpose(1, 2, 0, 3)
        aug = a32.astype(np.float16)
        # sequence DRAM as [group, channel-partition, chunk, batch, col] so
        # group loads read contiguous spans
        aug = np.ascontiguousarray(
            aug.reshape(NCH, 128, NGRP, GRP, AC).transpose(2, 1, 0, 3, 4)
        ).reshape(NGRP, 128, NCH * GRP * AC)
        in_maps.append({"aug": aug})
    return in_maps


def run(feature_map1, feature_map2, trace=False):
    in_maps = _prep_in_maps(feature_map1, feature_map2)
    nc = _get_nc()
    res = run_bass_kernel_spmd(nc, in_maps, core_ids=list(range(NCORE)), trace=trace)
    out = np.concatenate(
        [np.asarray(res.results[i]["out"]).reshape(BS) for i in range(NCORE)]
    ).astype(np.float32)
    return out, res


def kernel(feature_map1, feature_map2):
    out, _ = run(feature_map1, feature_map2, trace=False)
    return out



# revision 15
# speedup vs baseline: 1.5255x; 1.5255x over previous
"""DeepEMD Trainium2 kernel: batched 49x49 entropic-OT (Sinkhorn) similarity.

Strategy (8 NeuronCores, data-parallel over batch; 128 batches/core):
- Host prepacks, per (chunk j of 128 channels, batch b), an augmented matrix
  A = [P | Q | 1] (128 x 99) in fp16, sequenced in DRAM so every load DMA
  reads one contiguous span (identical layout to the original baseline).
- PE computes the full Gram G_b = A^T A (99x99, fp32 PSUM) with one
  accumulating fp16 matmul per channel chunk. G contains P^T Q, Q^T P, both
  diagonals, and the column sums (ones row): the similarity map, node norms
  and weight vectors are all cheap fixups of G.
- Batch-major transposition runs through a DRAM round trip: per group of 16
  batches, the Gram tiles are DVE-copied (fp32->fp16 cast) into a [99, 16*99]
  staging tile, DMA'd to DRAM with 99 big contiguous descriptors, then read
  back with strided APs that land exactly the regions phase 2 needs --
  qtp block, ptq block, diag(QtQ), diag(PtP), sums -- already batch-major
  ([batch on partitions, 49x50-padded rows]).  This replaces the baseline's
  per-batch [99,99]->[1,9801] SBUF flattens (descriptor-rate bound, ~190us).
- Phase 2 runs fully batch-major in fp16 (DVE tensor_tensor at 2x) with the
  49x49 maps padded to 50 cols (4B alignment for the 2x mode). Sinkhorn runs
  5 linear-domain Gauss-Seidel iterations, i-rows split DVE/GpSimd so both
  engines reduce concurrently. exp() on the ACT engine with a safe fp16 bias.
- logits = T * us^T((K.sim) vs) / s2, one fused tensor_tensor_reduce.
"""

import os
import sys

import numpy as np

BISECT = set(filter(None, os.environ.get("BISECT", "").split(",")))
# flags: nodma_b, nodiag, diag_gp, nogp, nottr

sys.path.insert(0, "/opt/trn_rl_repo")

import concourse.bass as bass
import concourse.bacc as bacc
import concourse.mybir as mybir
from concourse import tile
from concourse.bass_utils import run_bass_kernel_spmd

B_FULL, C, HW = 1024, 512, 49
NCORE = 8
BS = B_FULL // NCORE  # 128 batches per core
NCH = C // 128  # 4 chunks of 128 channels (PE contraction dim)
AC = 2 * HW + 1  # 99 augmented columns [P | Q | 1]
GRP = 16  # batches per group
NGRP = BS // GRP
JW = GRP * AC  # 1584 cols per chunk-slab in stage
HWP = HW + 1  # 50: padded row width for 4B-aligned fp16 rows
ITERS = 4
EPS_S = 0.05
TEMP = 12.5 / HW
EXP_BIAS = 10.0  # K = exp((sim-1)/eps + EXP_BIAS); cancels in us*K*vs
DV = 39  # Sinkhorn mul rows handled by DVE; rest on GpSimd (reduces: DVE only)
GPR = HW - DV

f32 = mybir.dt.float32
f16 = mybir.dt.float16
Alu = mybir.AluOpType
Act = mybir.ActivationFunctionType
AxX = mybir.AxisListType.X


def build_nc(debug=False):
    nc = bacc.Bacc(None, target_bir_lowering=False, debug=debug)
    GP = nc.vector if "nogp" in BISECT else nc.gpsimd
    aug = nc.declare_dram_parameter("aug", [NGRP, 128, NCH * JW], f16, isOutput=False)
    outp = nc.declare_dram_parameter("out", [BS, 1], f32, isOutput=True)

    with tile.TileContext(nc) as tc:
        with (
            tc.tile_pool(name="big", bufs=1) as big,
            tc.tile_pool(name="stage", bufs=3) as stg,
            tc.tile_pool(name="gstage", bufs=2) as gst,
            tc.tile_pool(name="small", bufs=1) as sml,
            tc.tile_pool(name="dram", bufs=1, space="DRAM") as dpool,
            tc.tile_pool(name="psum", bufs=8, space="PSUM") as pp,
        ):
            scr = dpool.tile([NGRP, AC, JW], f16, tag="scr", name="scr")

            # batch-major phase-2 inputs (fp16, rows padded to 50)
            qtpb = big.tile([BS, HW * HWP], f16, tag="qtpb", name="qtpb")
            ptqb = big.tile([BS, HW * HWP], f16, tag="ptqb", name="ptqb")
            dq16 = big.tile([BS, HW], f16, tag="dq16", name="dq16")
            dp16 = big.tile([BS, HW], f16, tag="dp16", name="dp16")
            sums16 = big.tile([BS, 2 * HW], f16, tag="sums16", name="sums16")

            def v3(t, w=HWP):  # [BS, r, c] view of flat tile
                return t[:].rearrange("p (q c) -> p q c", c=w)

            # pad cols of the DMA-written blocks are never written by DMA:
            # zero them so downstream fp16 math can't meet NaN junk
            if bool(BISECT & {"nodma_b", "nodiag"}):
                nc.vector.memset(qtpb[:], 0.01)
                nc.vector.memset(ptqb[:], 0.01)
                nc.vector.memset(sums16[:], 0.01)
                nc.vector.memset(dq16[:], 1.0)
                nc.vector.memset(dp16[:], 1.0)
            if "nodiag" in BISECT:
                nc.vector.memset(dq16[:], 1.0)
                nc.vector.memset(dp16[:], 1.0)
            nc.vector.memset(v3(qtpb)[:, :, HW:HWP], 0.0)
            nc.vector.memset(v3(ptqb)[:, :, HW:HWP], 0.0)

            # ---- early: zero pads, warm ACT tables (hidden under loads) ----
            ebias = sml.tile([BS, 1], f32, tag="ebias", name="ebias")
            nc.vector.memset(ebias[:], EXP_BIAS - 1.0 / EPS_S)
            wrm = sml.tile([BS, 1], f32, tag="wrm", name="wrm")
            nc.vector.memset(wrm[:], 1.0)
            nc.scalar.activation(wrm[:], wrm[:], Act.Sqrt)
            nc.scalar.activation(wrm[:], wrm[:], Act.Exp)

            # ---------------- Phase 1: load + Gram + DRAM round trip --------
            for g in range(NGRP):
                th = stg.tile([128, NCH * JW], f16, tag="h", name="hg")
                SW = NCH * JW // 2
                for ss in range(2):
                    nc.sync.dma_start(
                        th[:, ss * SW : (ss + 1) * SW],
                        aug[g, :, ss * SW : (ss + 1) * SW],
                    )
                stage = gst.tile([AC, JW], f16, tag="st", name="stage")
                for bb in range(GRP):
                    ps = pp.tile([AC, AC], f32, tag="gram", name="gram")
                    for j in range(NCH):
                        base = j * JW + bb * AC
                        nc.tensor.matmul(
                            ps[:, :],
                            th[:, base : base + AC],
                            th[:, base : base + AC],
                            start=(j == 0),
                            stop=(j == NCH - 1),
                        )
                    # PSUM fp32 -> fp16 staging (cast)
                    nc.vector.tensor_copy(stage[:, bb * AC : (bb + 1) * AC], ps[:, :])
                # stage -> DRAM: 99 contiguous 3168B descriptors
                if "noscr" not in BISECT:
                    nc.scalar.dma_start(scr[g], stage[:])
                # DRAM -> batch-major regions (strided reads, 16-part dests)
                b0 = g * GRP
                if "nodma_b" not in BISECT:
                    dst_q = v3(qtpb)[b0 : b0 + GRP, :, 0:HW]
                    src_q = scr[g, HW : 2 * HW, :].rearrange("r (b c) -> b r c", c=AC)[
                        :, :, 0:HW
                    ]
                    nc.sync.dma_start(dst_q, src_q)
                    dst_p = v3(ptqb)[b0 : b0 + GRP, :, 0:HW]
                    src_p = scr[g, 0:HW, :].rearrange("r (b c) -> b r c", c=AC)[
                        :, :, HW : 2 * HW
                    ]
                    nc.sync.dma_start(dst_p, src_p)
                    # sums row 98: [sp(49) | sq(49)]
                    src_s = scr[g, 2 * HW, :].rearrange("(b c) -> b c", c=AC)[
                        :, 0 : 2 * HW
                    ]
                    nc.scalar.dma_start(sums16[b0 : b0 + GRP, :], src_s)
                if not BISECT & {"nodma_b", "nodiag"}:
                    # diagQ[i] = G[49+i, 49+i]; diagP[i] = G[i, i]
                    diag_eng = nc.gpsimd if "diag_gp" in BISECT else nc.scalar
                    vdq = scr[g, HW : HW + 1, HW : HW + 1].copy()
                    vdq.ap = mybir.VecI64Pair([[AC, GRP], [JW + 1, HW], [1, 1]])
                    diag_eng.dma_start(dq16[b0 : b0 + GRP, :], vdq)
                    vdp = scr[g, 0:1, 0:1].copy()
                    vdp.ap = mybir.VecI64Pair([[AC, GRP], [JW + 1, HW], [1, 1]])
                    diag_eng.dma_start(dp16[b0 : b0 + GRP, :], vdp)

            # ---------------- Phase 1.5: fixups -> sim, K, weights ----------
            def s49(tag, dt=f32, w=HW):
                t = sml.tile([BS, w], dt, tag=tag, name=tag)
                return t

            dq32, dp32 = s49("dq32"), s49("dp32")
            s32 = s49("s32", w=2 * HW)
            nc.vector.tensor_copy(dq32[:], dq16[:])
            nc.scalar.copy(dp32[:], dp16[:])
            nc.scalar.copy(s32[:], sums16[:])
            sp, sq = s32[:, 0:HW], s32[:, HW : 2 * HW]

            t1, t2 = s49("t1"), s49("t2")
            inq, inp_ = s49("inq", w=HWP), s49("inp", w=HWP)
            aq, ap_ = s49("aq", w=HWP), s49("ap", w=HWP)
            nc.vector.memset(inq[:], 0.0)
            nc.vector.memset(inp_[:], 0.0)
            nc.vector.memset(aq[:], 0.0)
            nc.vector.memset(ap_[:], 0.0)
            for (sx, dx, inv) in ((sq, dq32, inq), (sp, dp32, inp_)):
                # inv = rsqrt(dx - sx^2/C) via sqrt LUT + exact recip + Newton
                nc.vector.tensor_mul(t1[:], sx, sx)
                nc.vector.scalar_tensor_tensor(
                    t2[:], t1[:], -1.0 / C, dx[:], Alu.mult, Alu.add
                )
                nc.scalar.activation(t1[:], t2[:], Act.Sqrt)
                nc.vector.reciprocal(inv[:, 0:HW], t1[:])
                nc.vector.tensor_mul(t1[:], inv[:, 0:HW], inv[:, 0:HW])
                nc.vector.tensor_mul(t1[:], t1[:], t2[:])
                nc.vector.tensor_scalar(t1[:], t1[:], -0.5, 1.5, Alu.mult, Alu.add)
                nc.vector.tensor_mul(inv[:, 0:HW], inv[:, 0:HW], t1[:])
            rC = 1.0 / np.sqrt(float(C))
            nc.vector.scalar_tensor_tensor(
                aq[:, 0:HW], sq, rC, inq[:, 0:HW], Alu.mult, Alu.mult
            )
            nc.vector.scalar_tensor_tensor(
                ap_[:, 0:HW], sp, rC, inp_[:, 0:HW], Alu.mult, Alu.mult
            )

            # weight vectors from raw qtp: w1[i]=relu(mean_j qtp)+1e-3 etc.
            w1r, w2r = s49("w1r"), s49("w2r")
            w1, w2 = s49("w1"), s49("w2")
            nc.vector.tensor_reduce(w1r[:], v3(qtpb), axis=AxX, op=Alu.add)
            nc.vector.tensor_reduce(w2r[:], v3(ptqb), axis=AxX, op=Alu.add)
            for (w, wr) in ((w1, w1r), (w2, w2r)):
                nc.vector.tensor_scalar(w[:], wr[:], 1.0 / HW, 0.0, Alu.mult, Alu.max)
                nc.vector.tensor_scalar(w[:], w[:], 0.001, None, Alu.add)
            s2 = sml.tile([BS, 1], f32, tag="s2", name="s2")
            rs2 = sml.tile([BS, 1], f32, tag="rs2", name="rs2")
            nc.vector.tensor_reduce(s2[:], w2[:], axis=AxX, op=Alu.add)
            nc.vector.reciprocal(rs2[:], s2[:])

            # b1 = inq x inp, b3 = aq x ap  ([50x50]-padded, row 49 zeroed)
            b1 = big.tile([BS, HWP * HWP], f16, tag="b1", name="b1")
            b3 = big.tile([BS, HWP * HWP], f16, tag="b3", name="b3")
            nc.vector.memset(b1[:, HW * HWP :], 0.0)
            GP.memset(b3[:, HW * HWP :], 0.0)
            bq = inq[:, 0:HW].unsqueeze(2).broadcast_to([BS, HW, HWP])
            bp = inp_[:].unsqueeze(1).broadcast_to([BS, HW, HWP])
            nc.vector.tensor_mul(v3(b1)[:, 0:HW, :], bq, bp)
            baq = aq[:, 0:HW].unsqueeze(2).broadcast_to([BS, HW, HWP])
            bap = ap_[:].unsqueeze(1).broadcast_to([BS, HW, HWP])
            GP.tensor_mul(v3(b3)[:, 0:HW, :], baq, bap)

            simb = big.tile([BS, HW * HWP], f16, tag="simb", name="simb")
            simTb = big.tile([BS, HW * HWP], f16, tag="simTb", name="simTb")
            Kb = big.tile([BS, HW * HWP], f16, tag="Kb", name="Kb")
            Ktb = big.tile([BS, HW * HWP], f16, tag="Ktb", name="Ktb")
            KSb = big.tile([BS, HW * HWP], f16, tag="KSb", name="KSb")
            nc.vector.tensor_mul(v3(simb), v3(qtpb), v3(b1)[:, 0:HW, :])
            nc.vector.tensor_sub(v3(simb), v3(simb), v3(b3)[:, 0:HW, :])
            # simT = transpose view copy of sim (per-lane 49x49 transpose)
            nc.vector.memset(v3(simTb)[:, :, HW:HWP], 0.0)
            stv = simb[:].rearrange("p (i j) -> p j i", j=HWP)[:, 0:HW, :]
            GP.tensor_copy(v3(simTb)[:, :, 0:HW], stv)
            nc.scalar.activation(Kb[:], simb[:], Act.Exp, scale=1.0 / EPS_S, bias=ebias[:])
            nc.scalar.activation(Ktb[:], simTb[:], Act.Exp, scale=1.0 / EPS_S, bias=ebias[:])
            # zero the padded col of K/Kt (sim pad rode junk-free zeros -> e^-10)
            nc.vector.memset(v3(Kb)[:, :, HW:HWP], 0.0)
            nc.vector.memset(v3(Ktb)[:, :, HW:HWP], 0.0)
            nc.vector.tensor_mul(KSb[:], Kb[:], simb[:])

            # ---------------- Phase 2: Sinkhorn (Gauss-Seidel, fp16) --------
            kv = s49("kv")
            rkv = s49("rkv")
            us16 = s49("us16", f16, w=HWP)
            vs16 = s49("vs16", f16, w=HWP)
            nc.vector.memset(us16[:], 0.0)
            nc.vector.memset(vs16[:], 0.0)
            tbA = big.tile([BS, DV * HWP], f16, tag="tbA", name="tbA")
            tbB = big.tile([BS, GPR * HWP], f16, tag="tbB", name="tbB")

            K3, Kt3, KS3 = v3(Kb), v3(Ktb), v3(KSb)
            vA3 = tbA[:].rearrange("p (q c) -> p q c", c=HWP)
            vB3 = tbB[:].rearrange("p (q c) -> p q c", c=HWP)

            def half_step(src3, bvec, wvec, dst16, first=False):
                bcA = bvec[:].unsqueeze(1).broadcast_to([BS, DV, HWP])
                bcB = bvec[:].unsqueeze(1).broadcast_to([BS, GPR, HWP])
                if first:
                    nc.vector.tensor_reduce(
                        kv[:, 0:HW], src3[:, 0:HW, :], axis=AxX, op=Alu.add
                    )
                else:
                    GP.tensor_mul(vB3, src3[:, DV:HW, :], bcB)
                    nc.vector.tensor_mul(vA3, src3[:, 0:DV, :], bcA)
                    nc.vector.tensor_reduce(kv[:, 0:DV], vA3, axis=AxX, op=Alu.add)
                    nc.vector.tensor_reduce(kv[:, DV:HW], vB3, axis=AxX, op=Alu.add)
                nc.vector.reciprocal(rkv[:], kv[:])
                nc.vector.tensor_mul(dst16[:, 0:HW], wvec[:], rkv[:])

            for it in range(ITERS):
                half_step(K3, vs16, w1, us16, first=(it == 0))
                half_step(Kt3, us16, w2, vs16)

            # ---------------- Phase 3: logits -------------------------------
            bcA = vs16[:].unsqueeze(1).broadcast_to([BS, DV, HWP])
            bcB = vs16[:].unsqueeze(1).broadcast_to([BS, GPR, HWP])
            GP.tensor_mul(vB3, KS3[:, DV:HW, :], bcB)
            nc.vector.tensor_mul(vA3, KS3[:, 0:DV, :], bcA)
            nc.vector.tensor_reduce(kv[:, 0:DV], vA3, axis=AxX, op=Alu.add)
            nc.vector.tensor_reduce(kv[:, DV:HW], vB3, axis=AxX, op=Alu.add)
            us32 = s49("us32")
            nc.vector.tensor_copy(us32[:], us16[:, 0:HW])
            junk = s49("junk")
            lg = sml.tile([BS, 1], f32, tag="lg", name="lg")
            lgf = sml.tile([BS, 1], f32, tag="lgf", name="lgf")
            if "nottr" in BISECT:
                nc.vector.tensor_mul(junk[:], kv[:], us32[:])
                nc.vector.tensor_reduce(lg[:], junk[:], axis=AxX, op=Alu.add)
            else:
                nc.vector.tensor_tensor_reduce(
                    junk[:], kv[:], us32[:], 1.0, 0.0, Alu.mult, Alu.add, accum_out=lg[:]
                )
            nc.vector.scalar_tensor_tensor(
                lgf[:], lg[:], TEMP, rs2[:], Alu.mult, Alu.mult
            )
            nc.sync.dma_start(outp[:, :], lgf[:])

    nc.compile()
    return nc


_NC = None


def _get_nc():
    global _NC
    if _NC is None:
        _NC = build_nc()
    return _NC


def _prep_in_maps(feature_map1, feature_map2):
    q = np.ascontiguousarray(np.asarray(feature_map1, dtype=np.float32)).reshape(
        B_FULL, C, HW
    )
    p = np.ascontiguousarray(np.asarray(feature_map2, dtype=np.float32)).reshape(
        B_FULL, C, HW
    )
    in_maps = []
    for i in range(NCORE):
        sl = slice(i * BS, (i + 1) * BS)
        a32 = np.empty((NCH, 128, BS, AC), np.float32)
        a32[..., AC - 1] = 1.0
        a32[..., 0:HW] = p[sl].reshape(BS, NCH, 128, HW).transpose(1, 2, 0, 3)
        a32[..., HW : 2 * HW] = q[sl].reshape(BS, NCH, 128, HW).transpose(1, 2, 0, 3)
        augm = a32.astype(np.float16)
        # sequence DRAM as [group, channel-partition, chunk, batch, col] so
        # group loads read contiguous spans
        augm = np.ascontiguousarray(
            augm.reshape(NCH, 128, NGRP, GRP, AC).transpose(2, 1, 0, 3, 4)
        ).reshape(NGRP, 128, NCH * GRP * AC)
        in_maps.append({"aug": augm})
    return in_maps


def run(feature_map1, feature_map2, trace=False):
    in_maps = _prep_in_maps(feature_map1, feature_map2)
    nc = _get_nc()
    res = run_bass_kernel_spmd(nc, in_maps, core_ids=list(range(NCORE)), trace=trace)
    out = np.concatenate(
        [np.asarray(res.results[i]["out"]).reshape(BS) for i in range(NCORE)]
    ).astype(np.float32)
    return out, res


def kernel(feature_map1, feature_map2):
    out, _ = run(feature_map1, feature_map2, trace=False)
    return out


# revision 21
# speedup vs baseline: 1.9312x; 1.2660x over previous
"""DeepEMD Trainium2 kernel: batched 49x49 entropic-OT (Sinkhorn) similarity.

Strategy (8 NeuronCores, data-parallel over batch; 128 batches/core):
- Host prepacks, per (chunk j of 128 channels, batch b), an augmented matrix
  A = [P | Q | 1] (128 x 99) in fp16, sequenced in DRAM so every load DMA
  reads one contiguous span (identical layout to the original baseline).
- PE computes the full Gram G_b = A^T A (99x99, fp32 PSUM) with one
  accumulating fp16 matmul per channel chunk. G contains P^T Q, Q^T P, both
  diagonals, and the column sums (ones row): the similarity map, node norms
  and weight vectors are all cheap fixups of G.
- Batch-major transposition runs through a DRAM round trip: per group of 16
  batches, the Gram tiles are DVE-copied (fp32->fp16 cast) into a [99, 16*99]
  staging tile, DMA'd to DRAM with 99 big contiguous descriptors, then read
  back with strided APs that land exactly the regions phase 2 needs --
  qtp block, ptq block, diag(QtQ), diag(PtP), sums -- already batch-major
  ([batch on partitions, 49x50-padded rows]).  This replaces the baseline's
  per-batch [99,99]->[1,9801] SBUF flattens (descriptor-rate bound, ~190us).
- Phase 2 runs fully batch-major in fp16 (DVE tensor_tensor at 2x) with the
  49x49 maps padded to 50 cols (4B alignment for the 2x mode). Sinkhorn runs
  5 linear-domain Gauss-Seidel iterations, i-rows split DVE/GpSimd so both
  engines reduce concurrently. exp() on the ACT engine with a safe fp16 bias.
- logits = T * us^T((K.sim) vs) / s2, one fused tensor_tensor_reduce.
"""

import os
import sys

import numpy as np

BISECT = set(filter(None, os.environ.get("BISECT", "").split(",")))
# flags: nodma_b, nodiag, diag_gp, nogp, nottr

sys.path.insert(0, "/opt/trn_rl_repo")

import concourse.bass as bass
import concourse.bacc as bacc
import concourse.mybir as mybir
from concourse import tile
from concourse.bass_utils import run_bass_kernel_spmd

B_FULL, C, HW = 1024, 512, 49
NCORE = 8
BS = B_FULL // NCORE  # 128 batches per core
NCH = C // 128  # 4 chunks of 128 channels (PE contraction dim)
AC = 2 * HW + 1  # 99 augmented columns [P | Q | 1]
GRP = 16  # batches per group
NGRP = BS // GRP
JW = GRP * AC  # 1584 cols per chunk-slab in stage
HWP = HW + 1  # 50: padded row width for 4B-aligned fp16 rows
ITERS = 4
EPS_S = 0.05
TEMP = 12.5 / HW
EXP_BIAS = 10.0  # K = exp((sim-1)/eps + EXP_BIAS); cancels in us*K*vs
DV = 39  # Sinkhorn mul rows handled by DVE; rest on GpSimd (reduces: DVE only)
GPR = HW - DV

f32 = mybir.dt.float32
f16 = mybir.dt.float16
Alu = mybir.AluOpType
Act = mybir.ActivationFunctionType
AxX = mybir.AxisListType.X


def build_nc(debug=False):
    nc = bacc.Bacc(None, target_bir_lowering=False, debug=debug)
    GP = nc.vector if "nogp" in BISECT else nc.gpsimd
    aug = nc.declare_dram_parameter("aug", [NGRP, 128, NCH * JW], f16, isOutput=False)
    diags = nc.declare_dram_parameter("diags", [BS, 4 * HW], f16, isOutput=False)
    outp = nc.declare_dram_parameter("out", [BS, 1], f32, isOutput=True)

    with tile.TileContext(nc) as tc:
        with (
            tc.tile_pool(name="big", bufs=1) as big,
            tc.tile_pool(name="stage", bufs=3) as stg,
            tc.tile_pool(name="gstage", bufs=2) as gst,
            tc.tile_pool(name="small", bufs=1) as sml,
            tc.tile_pool(name="dram", bufs=1, space="DRAM") as dpool,
            tc.tile_pool(name="psum", bufs=8, space="PSUM") as pp,
        ):
            scr = dpool.tile([NGRP, HWP, JW], f16, tag="scr", name="scr")

            # batch-major phase-2 inputs (fp16, rows padded to 50)
            qtpb = big.tile([BS, HW * HWP], f16, tag="qtpb", name="qtpb")
            dgs16 = big.tile([BS, 4 * HW], f16, tag="dgs16", name="dgs16")

            def v3(t, w=HWP):  # [BS, r, c] view of flat tile
                return t[:].rearrange("p (q c) -> p q c", c=w)

            # pad cols of the DMA-written blocks are never written by DMA:
            # zero them so downstream fp16 math can't meet NaN junk
            if bool(BISECT & {"nodma_b", "nodiag"}):
                nc.vector.memset(qtpb[:], 0.01)
            nc.vector.memset(v3(qtpb)[:, :, HW:HWP], 0.0)
            nc.sync.dma_start(dgs16[:], diags[:, :])

            # ---- early: zero pads, warm ACT tables (hidden under loads) ----
            ebias = sml.tile([BS, 1], f32, tag="ebias", name="ebias")
            nc.vector.memset(ebias[:], EXP_BIAS - 1.0 / EPS_S)
            wrm = sml.tile([BS, 1], f32, tag="wrm", name="wrm")
            nc.vector.memset(wrm[:], 1.0)
            nc.scalar.activation(wrm[:], wrm[:], Act.Sqrt)
            nc.scalar.activation(wrm[:], wrm[:], Act.Exp)

            # ---- fixups that depend only on host stats: run under loads ----
            def s49(tag, dt=f32, w=HW):
                t = sml.tile([BS, w], dt, tag=tag, name=tag)
                return t

            dq32, dp32 = s49("dq32"), s49("dp32")
            s32 = s49("s32", w=2 * HW)
            nc.vector.tensor_copy(dq32[:], dgs16[:, 0:HW])
            nc.scalar.copy(dp32[:], dgs16[:, HW : 2 * HW])
            nc.scalar.copy(s32[:], dgs16[:, 2 * HW : 4 * HW])
            sp, sq = s32[:, 0:HW], s32[:, HW : 2 * HW]

            t1, t2 = s49("t1"), s49("t2")
            inq, inp_ = s49("inq", w=HWP), s49("inp", w=HWP)
            aq, ap_ = s49("aq", w=HWP), s49("ap", w=HWP)
            nc.vector.memset(inq[:], 0.0)
            nc.vector.memset(inp_[:], 0.0)
            nc.vector.memset(aq[:], 0.0)
            nc.vector.memset(ap_[:], 0.0)
            for (sx, dx, inv) in ((sq, dq32, inq), (sp, dp32, inp_)):
                # inv = rsqrt(dx - sx^2/C) via sqrt LUT + exact recip + Newton
                nc.vector.tensor_mul(t1[:], sx, sx)
                nc.vector.scalar_tensor_tensor(
                    t2[:], t1[:], -1.0 / C, dx[:], Alu.mult, Alu.add
                )
                nc.scalar.activation(t1[:], t2[:], Act.Sqrt)
                nc.vector.reciprocal(inv[:, 0:HW], t1[:])
                nc.vector.tensor_mul(t1[:], inv[:, 0:HW], inv[:, 0:HW])
                nc.vector.tensor_mul(t1[:], t1[:], t2[:])
                nc.vector.tensor_scalar(t1[:], t1[:], -0.5, 1.5, Alu.mult, Alu.add)
                nc.vector.tensor_mul(inv[:, 0:HW], inv[:, 0:HW], t1[:])
            rC = 1.0 / np.sqrt(float(C))
            nc.vector.scalar_tensor_tensor(
                aq[:, 0:HW], sq, rC, inq[:, 0:HW], Alu.mult, Alu.mult
            )
            nc.vector.scalar_tensor_tensor(
                ap_[:, 0:HW], sp, rC, inp_[:, 0:HW], Alu.mult, Alu.mult
            )

            # b1 = inq x inp, b3 = aq x ap  ([50x50]-padded, row 49 zeroed)
            b1 = big.tile([BS, HWP * HWP], f16, tag="b1", name="b1")
            b3 = big.tile([BS, HWP * HWP], f16, tag="b3", name="b3")
            nc.vector.memset(b1[:, HW * HWP :], 0.0)
            GP.memset(b3[:, HW * HWP :], 0.0)
            bq = inq[:, 0:HW].unsqueeze(2).broadcast_to([BS, HW, HWP])
            bp = inp_[:].unsqueeze(1).broadcast_to([BS, HW, HWP])
            nc.vector.tensor_mul(v3(b1)[:, 0:HW, :], bq, bp)
            baq = aq[:, 0:HW].unsqueeze(2).broadcast_to([BS, HW, HWP])
            bap = ap_[:].unsqueeze(1).broadcast_to([BS, HW, HWP])
            GP.tensor_mul(v3(b3)[:, 0:HW, :], baq, bap)

            # ---------------- Phase 1: load + Gram + DRAM round trip --------
            for g in range(NGRP):
                th = stg.tile([128, NCH * JW], f16, tag="h", name="hg")
                SW = NCH * JW // 2
                for ss, eng in ((0, nc.sync), (1, nc.scalar)):
                    eng.dma_start(
                        th[:, ss * SW : (ss + 1) * SW],
                        aug[g, :, ss * SW : (ss + 1) * SW],
                    )
                stage = gst.tile([AC, JW], f16, tag="st", name="stage")
                for bb in range(GRP):
                    ps = pp.tile([AC, AC], f32, tag="gram", name="gram")
                    for j in range(NCH):
                        base = j * JW + bb * AC
                        nc.tensor.matmul(
                            ps[:, :],
                            th[:, base : base + AC],
                            th[:, base : base + AC],
                            start=(j == 0),
                            stop=(j == NCH - 1),
                        )
                    # PSUM fp32 -> fp16 staging (cast): rows 49:99 only
                    if bb % 2:
                        nc.vector.tensor_copy(
                            stage[:, bb * AC : (bb + 1) * AC], ps[:, :]
                        )
                    else:
                        nc.scalar.copy(stage[:, bb * AC : (bb + 1) * AC], ps[:, :])
                # stage rows 49:99 -> DRAM: 50 contiguous 3168B descriptors
                if "noscr" not in BISECT:
                    nc.scalar.dma_start(scr[g], stage[HW:AC, :])
                # DRAM -> batch-major regions (strided reads, 16-part dests)
                b0 = g * GRP
                if "nodma_b" not in BISECT:
                    SPL = 25
                    srcv = scr[g, 0:HW, :].rearrange("r (b c) -> b r c", c=AC)
                    nc.sync.dma_start(
                        v3(qtpb)[b0 : b0 + GRP, 0:SPL, 0:HW], srcv[:, 0:SPL, 0:HW]
                    )
                    nc.scalar.dma_start(
                        v3(qtpb)[b0 : b0 + GRP, SPL:HW, 0:HW], srcv[:, SPL:HW, 0:HW]
                    )

            # ---------------- Phase 1.5: fixups -> sim, K, weights ----------
            # weight vectors from raw qtp: w1[i]=relu(mean_j qtp)+1e-3 etc.
            w1r, w2r = s49("w1r"), s49("w2r")
            w1, w2 = s49("w1"), s49("w2")
            qtpT = qtpb[:].rearrange("p (i j) -> p j i", j=HWP)[:, 0:HW, :]
            nc.vector.tensor_reduce(w1r[:], v3(qtpb), axis=AxX, op=Alu.add)
            nc.vector.tensor_reduce(w2r[:], qtpT, axis=AxX, op=Alu.add)
            for (w, wr) in ((w1, w1r), (w2, w2r)):
                nc.vector.tensor_scalar(w[:], wr[:], 1.0 / HW, 0.0, Alu.mult, Alu.max)
                nc.vector.tensor_scalar(w[:], w[:], 0.001, None, Alu.add)
            s2 = sml.tile([BS, 1], f32, tag="s2", name="s2")
            rs2 = sml.tile([BS, 1], f32, tag="rs2", name="rs2")
            nc.vector.tensor_reduce(s2[:], w2[:], axis=AxX, op=Alu.add)
            nc.vector.reciprocal(rs2[:], s2[:])

            simb = big.tile([BS, HW * HWP], f16, tag="simb", name="simb")
            simTb = big.tile([BS, HW * HWP], f16, tag="simTb", name="simTb")
            Kb = big.tile([BS, HW * HWP], f16, tag="Kb", name="Kb")
            Ktb = big.tile([BS, HW * HWP], f16, tag="Ktb", name="Ktb")
            KSb = big.tile([BS, HW * HWP], f16, tag="KSb", name="KSb")
            nc.vector.tensor_mul(v3(simb), v3(qtpb), v3(b1)[:, 0:HW, :])
            nc.vector.tensor_sub(v3(simb), v3(simb), v3(b3)[:, 0:HW, :])
            # simT = transpose view copy of sim (per-lane 49x49 transpose)
            nc.vector.memset(v3(simTb)[:, :, HW:HWP], 0.0)
            stv = simb[:].rearrange("p (i j) -> p j i", j=HWP)[:, 0:HW, :]
            nc.scalar.copy(v3(simTb)[:, :, 0:HW], stv)
            nc.scalar.activation(Kb[:], simb[:], Act.Exp, scale=1.0 / EPS_S, bias=ebias[:])
            nc.scalar.activation(Ktb[:], simTb[:], Act.Exp, scale=1.0 / EPS_S, bias=ebias[:])
            # zero the padded col of K/Kt (sim pad rode junk-free zeros -> e^-10)
            nc.vector.memset(v3(Kb)[:, :, HW:HWP], 0.0)
            nc.vector.memset(v3(Ktb)[:, :, HW:HWP], 0.0)
            nc.vector.tensor_mul(KSb[:], Kb[:], simb[:])

            # ---------------- Phase 2: Sinkhorn (Gauss-Seidel, fp16) --------
            kv = s49("kv")
            rkv = s49("rkv")
            us16 = s49("us16", f16, w=HWP)
            vs16 = s49("vs16", f16, w=HWP)
            nc.vector.memset(us16[:], 0.0)
            nc.vector.memset(vs16[:], 0.0)
            tbA = big.tile([BS, DV * HWP], f16, tag="tbA", name="tbA")
            tbB = big.tile([BS, GPR * HWP], f16, tag="tbB", name="tbB")

            K3, Kt3, KS3 = v3(Kb), v3(Ktb), v3(KSb)
            vA3 = tbA[:].rearrange("p (q c) -> p q c", c=HWP)
            vB3 = tbB[:].rearrange("p (q c) -> p q c", c=HWP)

            def half_step(src3, bvec, wvec, dst16, first=False):
                bcA = bvec[:].unsqueeze(1).broadcast_to([BS, DV, HWP])
                bcB = bvec[:].unsqueeze(1).broadcast_to([BS, GPR, HWP])
                if first:
                    nc.vector.tensor_reduce(
                        kv[:, 0:HW], src3[:, 0:HW, :], axis=AxX, op=Alu.add
                    )
                else:
                    GP.tensor_mul(vB3, src3[:, DV:HW, :], bcB)
                    nc.vector.tensor_mul(vA3, src3[:, 0:DV, :], bcA)
                    nc.vector.tensor_reduce(kv[:, 0:DV], vA3, axis=AxX, op=Alu.add)
                    nc.vector.tensor_reduce(kv[:, DV:HW], vB3, axis=AxX, op=Alu.add)
                nc.vector.reciprocal(rkv[:], kv[:])
                nc.vector.tensor_mul(dst16[:, 0:HW], wvec[:], rkv[:])

            for it in range(ITERS):
                half_step(K3, vs16, w1, us16, first=(it == 0))
                half_step(Kt3, us16, w2, vs16)

            # ---------------- Phase 3: logits -------------------------------
            bcA = vs16[:].unsqueeze(1).broadcast_to([BS, DV, HWP])
            bcB = vs16[:].unsqueeze(1).broadcast_to([BS, GPR, HWP])
            GP.tensor_mul(vB3, KS3[:, DV:HW, :], bcB)
            nc.vector.tensor_mul(vA3, KS3[:, 0:DV, :], bcA)
            nc.vector.tensor_reduce(kv[:, 0:DV], vA3, axis=AxX, op=Alu.add)
            nc.vector.tensor_reduce(kv[:, DV:HW], vB3, axis=AxX, op=Alu.add)
            us32 = s49("us32")
            nc.vector.tensor_copy(us32[:], us16[:, 0:HW])
            junk = s49("junk")
            lg = sml.tile([BS, 1], f32, tag="lg", name="lg")
            lgf = sml.tile([BS, 1], f32, tag="lgf", name="lgf")
            if "nottr" in BISECT:
                nc.vector.tensor_mul(junk[:], kv[:], us32[:])
                nc.vector.tensor_reduce(lg[:], junk[:], axis=AxX, op=Alu.add)
            else:
                nc.vector.tensor_tensor_reduce(
                    junk[:], kv[:], us32[:], 1.0, 0.0, Alu.mult, Alu.add, accum_out=lg[:]
                )
            nc.vector.scalar_tensor_tensor(
                lgf[:], lg[:], TEMP, rs2[:], Alu.mult, Alu.mult
            )
            nc.sync.dma_start(outp[:, :], lgf[:])

    nc.compile()
    return nc


_NC = None


def _get_nc():
    global _NC
    if _NC is None:
        _NC = build_nc()
    return _NC


def _prep_in_maps(feature_map1, feature_map2):
    q = np.ascontiguousarray(np.asarray(feature_map1, dtype=np.float32)).reshape(
        B_FULL, C, HW
    )
    p = np.ascontiguousarray(np.asarray(feature_map2, dtype=np.float32)).reshape(
        B_FULL, C, HW
    )
    in_maps = []
    for i in range(NCORE):
        sl = slice(i * BS, (i + 1) * BS)
        a32 = np.empty((NCH, 128, BS, AC), np.float32)
        a32[..., AC - 1] = 1.0
        a32[..., 0:HW] = p[sl].reshape(BS, NCH, 128, HW).transpose(1, 2, 0, 3)
        a32[..., HW : 2 * HW] = q[sl].reshape(BS, NCH, 128, HW).transpose(1, 2, 0, 3)
        augm = a32.astype(np.float16)
        # node squared-norms of the fp16 data: [diagQ(49) | diagP(49)] fp16
        sq32 = augm.astype(np.float32) ** 2
        dsum = sq32.sum(axis=(0, 1))  # [BS, AC]
        dgs = np.concatenate(
            [dsum[:, HW : 2 * HW], dsum[:, 0:HW]], axis=1
        ).astype(np.float16)
        # sequence DRAM as [group, channel-partition, chunk, batch, col] so
        # group loads read contiguous spans
        augm = np.ascontiguousarray(
            augm.reshape(NCH, 128, NGRP, GRP, AC).transpose(2, 1, 0, 3, 4)
        ).reshape(NGRP, 128, NCH * GRP * AC)
        in_maps.append({"aug": augm, "diags": dgs})
    return in_maps


def run(feature_map1, feature_map2, trace=False):
    in_maps = _prep_in_maps(feature_map1, feature_map2)
    nc = _get_nc()
    res = run_bass_kernel_spmd(nc, in_maps, core_ids=list(range(NCORE)), trace=trace)
    out = np.concatenate(
        [np.asarray(res.results[i]["out"]).reshape(BS) for i in range(NCORE)]
    ).astype(np.float32)
    return out, res


def kernel(feature_map1, feature_map2):
    out, _ = run(feature_map1, feature_map2, trace=False)
    return out


# revision 25
# speedup vs baseline: 2.1573x; 1.1171x over previous
"""DeepEMD Trainium2 kernel: batched 49x49 entropic-OT (Sinkhorn) similarity.

Strategy (8 NeuronCores, data-parallel over batch; 128 batches/core):
- Host prepacks, per (chunk j of 128 channels, batch b), an augmented matrix
  A = [P | Q | 1] (128 x 99) in fp16, sequenced in DRAM so every load DMA
  reads one contiguous span (identical layout to the original baseline).
- PE computes the full Gram G_b = A^T A (99x99, fp32 PSUM) with one
  accumulating fp16 matmul per channel chunk. G contains P^T Q, Q^T P, both
  diagonals, and the column sums (ones row): the similarity map, node norms
  and weight vectors are all cheap fixups of G.
- Batch-major transposition runs through a DRAM round trip: per group of 16
  batches, the Gram tiles are DVE-copied (fp32->fp16 cast) into a [99, 16*99]
  staging tile, DMA'd to DRAM with 99 big contiguous descriptors, then read
  back with strided APs that land exactly the regions phase 2 needs --
  qtp block, ptq block, diag(QtQ), diag(PtP), sums -- already batch-major
  ([batch on partitions, 49x50-padded rows]).  This replaces the baseline's
  per-batch [99,99]->[1,9801] SBUF flattens (descriptor-rate bound, ~190us).
- Phase 2 runs fully batch-major in fp16 (DVE tensor_tensor at 2x) with the
  49x49 maps padded to 50 cols (4B alignment for the 2x mode). Sinkhorn runs
  5 linear-domain Gauss-Seidel iterations, i-rows split DVE/GpSimd so both
  engines reduce concurrently. exp() on the ACT engine with a safe fp16 bias.
- logits = T * us^T((K.sim) vs) / s2, one fused tensor_tensor_reduce.
"""

import os
import sys

import numpy as np

BISECT = set(filter(None, os.environ.get("BISECT", "").split(",")))
# opt-in experiment flags: "gp" (use GpSimd engine), "ttr" (fused final dot);
# bisect flags: nodma_b, nodiag, noscr

sys.path.insert(0, "/opt/trn_rl_repo")

import concourse.bass as bass
import concourse.bacc as bacc
import concourse.mybir as mybir
from concourse import tile
from concourse.bass_utils import run_bass_kernel_spmd

B_FULL, C, HW = 1024, 512, 49
NCORE = 8
BS = B_FULL // NCORE  # 128 batches per core
NCH = C // 128  # 4 chunks of 128 channels (PE contraction dim)
AC = 2 * HW + 1  # 99 augmented columns [P | Q | 1]
GRP = 16  # batches per group
NGRP = BS // GRP
JW = GRP * AC  # 1584 cols per chunk-slab in stage
HWP = HW + 1  # 50: padded row width for 4B-aligned fp16 rows
ITERS = 4
EPS_S = 0.05
TEMP = 12.5 / HW
EXP_BIAS = 10.0  # K = exp((sim-1)/eps + EXP_BIAS); cancels in us*K*vs
DV = 39  # Sinkhorn mul rows handled by DVE; rest on GpSimd (reduces: DVE only)
GPR = HW - DV

f32 = mybir.dt.float32
f16 = mybir.dt.float16
Alu = mybir.AluOpType
Act = mybir.ActivationFunctionType
AxX = mybir.AxisListType.X


def build_nc(debug=False):
    nc = bacc.Bacc(None, target_bir_lowering=False, debug=debug)
    GP = nc.gpsimd if "gp" in BISECT else nc.vector
    aug = nc.declare_dram_parameter("aug", [NGRP, 128, NCH * JW], f16, isOutput=False)
    diags = nc.declare_dram_parameter("diags", [BS, 4 * HW], f16, isOutput=False)
    outp = nc.declare_dram_parameter("out", [BS, 1], f32, isOutput=True)

    with tile.TileContext(nc) as tc:
        with (
            tc.tile_pool(name="big", bufs=1) as big,
            tc.tile_pool(name="stage", bufs=3) as stg,
            tc.tile_pool(name="gstage", bufs=2) as gst,
            tc.tile_pool(name="small", bufs=1) as sml,
            tc.tile_pool(name="dram", bufs=1, space="DRAM") as dpool,
            tc.tile_pool(name="psum", bufs=8, space="PSUM") as pp,
        ):
            scr = dpool.tile([NGRP, HWP, JW], f16, tag="scr", name="scr")

            # batch-major phase-2 inputs (fp16, rows padded to 50)
            qtpb = big.tile([BS, HW * HWP], f16, tag="qtpb", name="qtpb")
            dgs16 = big.tile([BS, 4 * HW], f16, tag="dgs16", name="dgs16")

            def v3(t, w=HWP):  # [BS, r, c] view of flat tile
                return t[:].rearrange("p (q c) -> p q c", c=w)

            # pad cols of the DMA-written blocks are never written by DMA:
            # zero them so downstream fp16 math can't meet NaN junk
            if bool(BISECT & {"nodma_b", "nodiag"}):
                nc.vector.memset(qtpb[:], 0.01)
            nc.vector.memset(v3(qtpb)[:, :, HW:HWP], 0.0)
            nc.sync.dma_start(dgs16[:], diags[:, :])

            # ---- early: zero pads, warm ACT tables (hidden under loads) ----
            ebias = sml.tile([BS, 1], f32, tag="ebias", name="ebias")
            nc.vector.memset(ebias[:], EXP_BIAS - 1.0 / EPS_S)
            wrm = sml.tile([BS, 1], f32, tag="wrm", name="wrm")
            nc.vector.memset(wrm[:], 1.0)
            nc.scalar.activation(wrm[:], wrm[:], Act.Sqrt)
            nc.scalar.activation(wrm[:], wrm[:], Act.Exp)

            # ---- fixups that depend only on host stats: run under loads ----
            def s49(tag, dt=f32, w=HW):
                t = sml.tile([BS, w], dt, tag=tag, name=tag)
                return t

            dq32, dp32 = s49("dq32"), s49("dp32")
            s32 = s49("s32", w=2 * HW)
            nc.vector.tensor_copy(dq32[:], dgs16[:, 0:HW])
            nc.scalar.copy(dp32[:], dgs16[:, HW : 2 * HW])
            nc.scalar.copy(s32[:], dgs16[:, 2 * HW : 4 * HW])
            sp, sq = s32[:, 0:HW], s32[:, HW : 2 * HW]

            t1, t2 = s49("t1"), s49("t2")
            inq, inp_ = s49("inq", w=HWP), s49("inp", w=HWP)
            aq, ap_ = s49("aq", w=HWP), s49("ap", w=HWP)
            nc.vector.memset(inq[:], 0.0)
            nc.vector.memset(inp_[:], 0.0)
            nc.vector.memset(aq[:], 0.0)
            nc.vector.memset(ap_[:], 0.0)
            for (sx, dx, inv) in ((sq, dq32, inq), (sp, dp32, inp_)):
                # inv = rsqrt(dx - sx^2/C) via sqrt LUT + exact recip + Newton
                nc.vector.tensor_mul(t1[:], sx, sx)
                nc.vector.scalar_tensor_tensor(
                    t2[:], t1[:], -1.0 / C, dx[:], Alu.mult, Alu.add
                )
                nc.scalar.activation(t1[:], t2[:], Act.Sqrt)
                nc.vector.reciprocal(inv[:, 0:HW], t1[:])
                nc.vector.tensor_mul(t1[:], inv[:, 0:HW], inv[:, 0:HW])
                nc.vector.tensor_mul(t1[:], t1[:], t2[:])
                nc.vector.tensor_scalar(t1[:], t1[:], -0.5, 1.5, Alu.mult, Alu.add)
                nc.vector.tensor_mul(inv[:, 0:HW], inv[:, 0:HW], t1[:])
            rC = 1.0 / np.sqrt(float(C))
            nc.vector.scalar_tensor_tensor(
                aq[:, 0:HW], sq, rC, inq[:, 0:HW], Alu.mult, Alu.mult
            )
            nc.vector.scalar_tensor_tensor(
                ap_[:, 0:HW], sp, rC, inp_[:, 0:HW], Alu.mult, Alu.mult
            )

            # b1 = inq x inp, b3 = aq x ap  ([50x50]-padded, row 49 zeroed)
            b1 = big.tile([BS, HWP * HWP], f16, tag="b1", name="b1")
            b3 = big.tile([BS, HWP * HWP], f16, tag="b3", name="b3")
            nc.vector.memset(b1[:, HW * HWP :], 0.0)
            GP.memset(b3[:, HW * HWP :], 0.0)
            bq = inq[:, 0:HW].unsqueeze(2).broadcast_to([BS, HW, HWP])
            bp = inp_[:].unsqueeze(1).broadcast_to([BS, HW, HWP])
            nc.vector.tensor_mul(v3(b1)[:, 0:HW, :], bq, bp)
            baq = aq[:, 0:HW].unsqueeze(2).broadcast_to([BS, HW, HWP])
            bap = ap_[:].unsqueeze(1).broadcast_to([BS, HW, HWP])
            GP.tensor_mul(v3(b3)[:, 0:HW, :], baq, bap)

            # ---------------- Phase 1: load + Gram + DRAM round trip --------
            for g in range(NGRP):
                th = stg.tile([128, NCH * JW], f16, tag="h", name="hg")
                SW = NCH * JW // 2
                for ss in range(2):
                    nc.sync.dma_start(
                        th[:, ss * SW : (ss + 1) * SW],
                        aug[g, :, ss * SW : (ss + 1) * SW],
                    )
                stage = gst.tile([AC, JW], f16, tag="st", name="stage")
                for bb in range(GRP):
                    ps = pp.tile([AC, AC], f32, tag="gram", name="gram")
                    for j in range(NCH):
                        base = j * JW + bb * AC
                        nc.tensor.matmul(
                            ps[:, :],
                            th[:, base : base + AC],
                            th[:, base : base + AC],
                            start=(j == 0),
                            stop=(j == NCH - 1),
                        )
                    # PSUM fp32 -> fp16 staging (cast): rows 49:99 only
                    if bb % 2:
                        nc.vector.tensor_copy(
                            stage[:, bb * AC : (bb + 1) * AC], ps[:, :]
                        )
                    else:
                        nc.scalar.copy(stage[:, bb * AC : (bb + 1) * AC], ps[:, :])
                # stage rows 49:99 -> DRAM: 50 contiguous 3168B descriptors
                if "noscr" not in BISECT:
                    nc.scalar.dma_start(scr[g], stage[HW:AC, :])
                # DRAM -> batch-major regions (strided reads, 16-part dests)
                b0 = g * GRP
                if "nodma_b" not in BISECT:
                    srcv = scr[g, 0:HW, :].rearrange("r (b c) -> b r c", c=AC)
                    nc.scalar.dma_start(
                        v3(qtpb)[b0 : b0 + GRP, :, 0:HW], srcv[:, :, 0:HW]
                    )

            # ---------------- Phase 1.5: fixups -> sim, K, weights ----------
            # weight vectors from raw qtp: w1[i]=relu(mean_j qtp)+1e-3 etc.
            w1r, w2r = s49("w1r"), s49("w2r")
            w1, w2 = s49("w1"), s49("w2")
            qtpT = qtpb[:].rearrange("p (i j) -> p j i", j=HWP)[:, 0:HW, :]
            nc.vector.tensor_reduce(w1r[:], v3(qtpb), axis=AxX, op=Alu.add)
            nc.vector.tensor_reduce(w2r[:], qtpT, axis=AxX, op=Alu.add)
            for (w, wr) in ((w1, w1r), (w2, w2r)):
                nc.vector.tensor_scalar(w[:], wr[:], 1.0 / HW, 0.0, Alu.mult, Alu.max)
                nc.vector.tensor_scalar(w[:], w[:], 0.001, None, Alu.add)
            s2 = sml.tile([BS, 1], f32, tag="s2", name="s2")
            rs2 = sml.tile([BS, 1], f32, tag="rs2", name="rs2")
            nc.vector.tensor_reduce(s2[:], w2[:], axis=AxX, op=Alu.add)
            nc.vector.reciprocal(rs2[:], s2[:])

            simb = big.tile([BS, HW * HWP], f16, tag="simb", name="simb")
            simTb = big.tile([BS, HW * HWP], f16, tag="simTb", name="simTb")
            Kb = big.tile([BS, HW * HWP], f16, tag="Kb", name="Kb")
            Ktb = big.tile([BS, HW * HWP], f16, tag="Ktb", name="Ktb")
            KSb = big.tile([BS, HW * HWP], f16, tag="KSb", name="KSb")
            nc.vector.tensor_mul(v3(simb), v3(qtpb), v3(b1)[:, 0:HW, :])
            nc.vector.tensor_sub(v3(simb), v3(simb), v3(b3)[:, 0:HW, :])
            # simT = transpose view copy of sim (per-lane 49x49 transpose)
            nc.vector.memset(v3(simTb)[:, :, HW:HWP], 0.0)
            stv = simb[:].rearrange("p (i j) -> p j i", j=HWP)[:, 0:HW, :]
            nc.scalar.copy(v3(simTb)[:, :, 0:HW], stv)
            nc.scalar.activation(Kb[:], simb[:], Act.Exp, scale=1.0 / EPS_S, bias=ebias[:])
            nc.scalar.activation(Ktb[:], simTb[:], Act.Exp, scale=1.0 / EPS_S, bias=ebias[:])
            # zero the padded col of K/Kt (sim pad rode junk-free zeros -> e^-10)
            nc.vector.memset(v3(Kb)[:, :, HW:HWP], 0.0)
            nc.vector.memset(v3(Ktb)[:, :, HW:HWP], 0.0)
            nc.vector.tensor_mul(KSb[:], Kb[:], simb[:])

            # ---------------- Phase 2: Sinkhorn (Gauss-Seidel, fp16) --------
            kv = s49("kv")
            rkv = s49("rkv")
            us16 = s49("us16", f16, w=HWP)
            vs16 = s49("vs16", f16, w=HWP)
            nc.vector.memset(us16[:], 0.0)
            nc.vector.memset(vs16[:], 0.0)
            tbA = big.tile([BS, DV * HWP], f16, tag="tbA", name="tbA")
            tbB = big.tile([BS, GPR * HWP], f16, tag="tbB", name="tbB")

            K3, Kt3, KS3 = v3(Kb), v3(Ktb), v3(KSb)
            cs, rcs = s49("cs"), s49("rcs")
            cssum = sml.tile([BS, 1], f32, tag="cssum", name="cssum")
            vA3 = tbA[:].rearrange("p (q c) -> p q c", c=HWP)
            vB3 = tbB[:].rearrange("p (q c) -> p q c", c=HWP)

            def half_step(src3, bvec, wvec, dst16, first=False):
                bcA = bvec[:].unsqueeze(1).broadcast_to([BS, DV, HWP])
                bcB = bvec[:].unsqueeze(1).broadcast_to([BS, GPR, HWP])
                if first:
                    nc.vector.tensor_reduce(
                        kv[:, 0:HW], src3[:, 0:HW, :], axis=AxX, op=Alu.add
                    )
                else:
                    GP.tensor_mul(vB3, src3[:, DV:HW, :], bcB)
                    nc.vector.tensor_mul(vA3, src3[:, 0:DV, :], bcA)
                    nc.vector.tensor_reduce(kv[:, 0:DV], vA3, axis=AxX, op=Alu.add)
                    nc.vector.tensor_reduce(kv[:, DV:HW], vB3, axis=AxX, op=Alu.add)
                nc.vector.reciprocal(rkv[:], kv[:])
                nc.vector.tensor_mul(dst16[:, 0:HW], wvec[:], rkv[:])

            if "it3" in BISECT:
                # warm start: vs0 = w2 * (sum cs / 49) / cs, cs = colsum(K)
                nc.vector.tensor_reduce(cs[:], Kt3, axis=AxX, op=Alu.add)
                nc.vector.tensor_reduce(cssum[:], cs[:], axis=AxX, op=Alu.add)
                nc.vector.reciprocal(rcs[:], cs[:])
                nc.vector.tensor_mul(rcs[:], w2[:], rcs[:])
                nc.vector.scalar_tensor_tensor(
                    vs16[:, 0:HW],
                    rcs[:],
                    1.0 / HW,
                    cssum[:].broadcast_to([BS, HW]),
                    Alu.mult,
                    Alu.mult,
                )
                for it in range(3):
                    half_step(K3, vs16, w1, us16)
                    half_step(Kt3, us16, w2, vs16)
            else:
                for it in range(ITERS):
                    half_step(K3, vs16, w1, us16, first=(it == 0))
                    half_step(Kt3, us16, w2, vs16)

            # ---------------- Phase 3: logits -------------------------------
            bcA = vs16[:].unsqueeze(1).broadcast_to([BS, DV, HWP])
            bcB = vs16[:].unsqueeze(1).broadcast_to([BS, GPR, HWP])
            GP.tensor_mul(vB3, KS3[:, DV:HW, :], bcB)
            nc.vector.tensor_mul(vA3, KS3[:, 0:DV, :], bcA)
            nc.vector.tensor_reduce(kv[:, 0:DV], vA3, axis=AxX, op=Alu.add)
            nc.vector.tensor_reduce(kv[:, DV:HW], vB3, axis=AxX, op=Alu.add)
            us32 = s49("us32")
            nc.vector.tensor_copy(us32[:], us16[:, 0:HW])
            junk = s49("junk")
            lg = sml.tile([BS, 1], f32, tag="lg", name="lg")
            lgf = sml.tile([BS, 1], f32, tag="lgf", name="lgf")
            if "ttr" in BISECT:
                nc.vector.tensor_tensor_reduce(
                    junk[:], kv[:], us32[:], 1.0, 0.0, Alu.mult, Alu.add, accum_out=lg[:]
                )
            else:
                nc.vector.tensor_mul(junk[:], kv[:], us32[:])
                nc.vector.tensor_reduce(lg[:], junk[:], axis=AxX, op=Alu.add)
            nc.vector.scalar_tensor_tensor(
                lgf[:], lg[:], TEMP, rs2[:], Alu.mult, Alu.mult
            )
            nc.sync.dma_start(outp[:, :], lgf[:])

    nc.compile()
    return nc


_NC = None


def _get_nc():
    global _NC
    if _NC is None:
        _NC = build_nc()
    return _NC


def _prep_in_maps(feature_map1, feature_map2):
    q = np.ascontiguousarray(np.asarray(feature_map1, dtype=np.float32)).reshape(
        B_FULL, C, HW
    )
    p = np.ascontiguousarray(np.asarray(feature_map2, dtype=np.float32)).reshape(
        B_FULL, C, HW
    )
    in_maps = []
    for i in range(NCORE):
        sl = slice(i * BS, (i + 1) * BS)
        a32 = np.empty((NCH, 128, BS, AC), np.float32)
        a32[..., AC - 1] = 1.0
        a32[..., 0:HW] = p[sl].reshape(BS, NCH, 128, HW).transpose(1, 2, 0, 3)
        a32[..., HW : 2 * HW] = q[sl].reshape(BS, NCH, 128, HW).transpose(1, 2, 0, 3)
        augm = a32.astype(np.float16)
        # host stats of the fp16 data: [diagQ | diagP | sp | sq] fp16
        a32f = augm.astype(np.float32)
        dsum = (a32f**2).sum(axis=(0, 1))  # [BS, AC] node squared-norms
        csum = a32f.sum(axis=(0, 1))  # [BS, AC] channel sums
        dgs = np.concatenate(
            [
                dsum[:, HW : 2 * HW],
                dsum[:, 0:HW],
                csum[:, 0:HW],
                csum[:, HW : 2 * HW],
            ],
            axis=1,
        ).astype(np.float16)
        # sequence DRAM as [group, channel-partition, chunk, batch, col] so
        # group loads read contiguous spans
        augm = np.ascontiguousarray(
            augm.reshape(NCH, 128, NGRP, GRP, AC).transpose(2, 1, 0, 3, 4)
        ).reshape(NGRP, 128, NCH * GRP * AC)
        in_maps.append({"aug": augm, "diags": dgs})
    return in_maps


def run(feature_map1, feature_map2, trace=False):
    in_maps = _prep_in_maps(feature_map1, feature_map2)
    nc = _get_nc()
    res = run_bass_kernel_spmd(nc, in_maps, core_ids=list(range(NCORE)), trace=trace)
    out = np.concatenate(
        [np.asarray(res.results[i]["out"]).reshape(BS) for i in range(NCORE)]
    ).astype(np.float32)
    return out, res


def kernel(feature_map1, feature_map2):
    out, _ = run(feature_map1, feature_map2, trace=False)
    return out
